# revision 8
# baseline (speedup 1.0000x reference)
"""Trainium2 Bass kernel for nn_FMCTracker.

Reference computation:
    xd   = dct2(x)                 # 2D DCT-II (ortho) over (H, W)
    gray = xd.mean(axis=1)         # channel mean
    w    = gray * attention
    row_out[b] = mean_h(rw[h] * mean_w(w[b,h,w]))
    col_out[b] = mean_w(cw[w] * mean_h(w[b,h,w]))
    out = sigmoid(stack([col_out, row_out], axis=1))   # [B, 2]

Everything between x and the sigmoid is linear in x, so the whole chain
folds into two fixed 512x512 weighting matrices (computed on the host from
the tiny attention/row/col weights):

    M_row = D^T @ (diag(rw) @ att) @ D / (C*H*W)
    M_col = D^T @ (att @ diag(cw)) @ D / (C*H*W)
    row_out[b] = sum_{c,i,j} x[b,c,i,j] * M_row[i,j]

The device kernel is then a pure memory-bound weighted reduction over x
(201 MB), data-parallel over batch across 8 NeuronCores (8 images each).

Per core, per image b:
    DMA   x[b] (3 MB fp32) -> SBUF as [128p, c, k, 512] (i = k*128 + p)
    DVE   S = c0 + c1 + c2              (two fp32 tensor_adds, bf16 out)
    DVE   tensor_tensor_reduce: (S * M_col) -> accum [128,1] fp32
    DVE   tensor_tensor_reduce: (S * M_row) -> accum [128,1] fp32
    (accum scalars collect into acc[128, 16], one column per (b, output))
Tail: one 128x16 matmul vs ones reduces partitions -> [16,1] PSUM,
ScalarE sigmoid, DMA 64 B out.
"""

import os
from contextlib import ExitStack

import numpy as np
import ml_dtypes

B, C, H, W = 64, 3, 512, 512
N_CORES = 8
B_PER_CORE = B // N_CORES  # 8


def _dct_matrix(n: int) -> np.ndarray:
    """DCT-II orthonormal matrix D, so that dct(v, norm='ortho') = D @ v."""
    i = np.arange(n)
    k = np.arange(n)[:, None]
    d = np.cos(np.pi * (2 * i[None, :] + 1) * k / (2 * n))
    coef = np.full((n, 1), np.sqrt(2.0 / n))
    coef[0] = np.sqrt(1.0 / n)
    return d * coef


def _weight_matrices(attention, row_weights, col_weights):
    """Fold DCT + all downstream linear reductions into two [H, W] matrices."""
    d = _dct_matrix(H)
    att = attention.astype(np.float64)
    rw = row_weights.astype(np.float64)
    cw = col_weights.astype(np.float64)
    scale = 1.0 / (C * H * W)
    m_row = d.T @ (rw[:, None] * att) @ d * scale
    m_col = d.T @ (att * cw[None, :]) @ d * scale
    return (
        m_row.astype(ml_dtypes.bfloat16),
        m_col.astype(ml_dtypes.bfloat16),
    )


_NC_CACHE = {}


def _build_bass():
    import concourse.bacc as bacc
    import concourse.tile as tile
    from concourse import mybir

    nc = bacc.Bacc(
        "TRN2", target_bir_lowering=False, debug=False, num_devices=N_CORES
    )
    x_in = nc.declare_dram_parameter(
        "x", [B_PER_CORE, C, H, W], mybir.dt.float32, isOutput=False
    )
    mrow_in = nc.declare_dram_parameter(
        "mrow", [H, W], mybir.dt.bfloat16, isOutput=False
    )
    mcol_in = nc.declare_dram_parameter(
        "mcol", [H, W], mybir.dt.bfloat16, isOutput=False
    )
    out_ext = nc.declare_dram_parameter(
        "out", [B_PER_CORE, 2], mybir.dt.float32, isOutput=True
    )

    P = 128
    K = H // P  # 4 chunks of rows per plane

    with tile.TileContext(nc) as tc:
        with ExitStack() as ctx:
            singles = ctx.enter_context(tc.tile_pool(name="singles", bufs=1))
            xpool = ctx.enter_context(tc.tile_pool(name="xpool", bufs=3))
            tpool = ctx.enter_context(tc.tile_pool(name="tpool", bufs=2))
            spool = ctx.enter_context(tc.tile_pool(name="spool", bufs=2))
            scrpool = ctx.enter_context(tc.tile_pool(name="scrpool", bufs=2))
            psum = ctx.enter_context(tc.tile_pool(name="psum", bufs=1, space="PSUM"))

            mrow_sb = singles.tile([P, K, W], mybir.dt.bfloat16)
            nc.sync.dma_start(
                out=mrow_sb, in_=mrow_in.rearrange("(k p) j -> p k j", p=P)
            )
            mcol_sb = singles.tile([P, K, W], mybir.dt.bfloat16)
            nc.sync.dma_start(
                out=mcol_sb, in_=mcol_in.rearrange("(k p) j -> p k j", p=P)
            )
            ones_sb = singles.tile([P, 1], mybir.dt.float32)
            nc.vector.memset(ones_sb, 1.0)
            acc = singles.tile([P, 2 * B_PER_CORE], mybir.dt.float32)

            for b in range(B_PER_CORE):
                xb = xpool.tile([P, C, K, W], mybir.dt.float32)
                nc.sync.dma_start(
                    out=xb, in_=x_in[b].rearrange("c (k p) j -> p c k j", p=P)
                )
                t01 = tpool.tile([P, K, W], mybir.dt.float32)
                nc.vector.tensor_add(t01, xb[:, 0], xb[:, 1])
                s_bf = spool.tile([P, K, W], mybir.dt.bfloat16)
                nc.vector.tensor_add(s_bf, t01, xb[:, 2])
                scr0 = scrpool.tile([P, K, W], mybir.dt.bfloat16, tag="scr")
                nc.vector.scalar_tensor_tensor(
                    out=scr0,
                    in0=s_bf,
                    scalar=0.0,
                    in1=mcol_sb,
                    op0=mybir.AluOpType.bypass,
                    op1=mybir.AluOpType.mult,
                    accum_out=acc[:, 2 * b : 2 * b + 1],
                )
                scr1 = scrpool.tile([P, K, W], mybir.dt.bfloat16, tag="scr")
                nc.vector.scalar_tensor_tensor(
                    out=scr1,
                    in0=s_bf,
                    scalar=0.0,
                    in1=mrow_sb,
                    op0=mybir.AluOpType.bypass,
                    op1=mybir.AluOpType.mult,
                    accum_out=acc[:, 2 * b + 1 : 2 * b + 2],
                )

            ps = psum.tile([2 * B_PER_CORE, 1], mybir.dt.float32)
            nc.tensor.matmul(
                out=ps, lhsT=acc, rhs=ones_sb, start=True, stop=True
            )
            out_sb = singles.tile([2 * B_PER_CORE, 1], mybir.dt.float32)
            nc.scalar.activation(
                out=out_sb, in_=ps, func=mybir.ActivationFunctionType.Sigmoid
            )
            nc.sync.dma_start(
                out=out_ext.rearrange("b e -> (b e) ()"), in_=out_sb
            )
    nc.compile()
    return nc


def _get_nc():
    if "nc" not in _NC_CACHE:
        _NC_CACHE["nc"] = _build_bass()
    return _NC_CACHE["nc"]


def _install_axon_ntff_shim():
    """Provide antenv.axon_hooks (NTFF profiling hook) when the image lacks it.

    Mirrors trn_agent_boot's ctypes hook against libaxon_pjrt.so so that
    run_bass_kernel_spmd(trace=True) can capture NTFF profiles under axon.
    """
    import sys
    import types
    import ctypes
    import contextlib

    try:
        from antenv.axon_hooks import get_axon_ntff_profile_hook  # noqa: F401

        return
    except ImportError:
        pass

    import antenv

    mod = types.ModuleType("antenv.axon_hooks")
    _state = {"hook": None}
    mod.set_axon_ntff_profile_hook = lambda h: _state.__setitem__("hook", h)
    mod.get_axon_ntff_profile_hook = lambda: _state["hook"]
    antenv.axon_hooks = mod
    sys.modules["antenv.axon_hooks"] = mod

    so_path = "/opt/axon/libaxon_pjrt.so"
    if not os.path.exists(so_path):
        return
    lib = ctypes.CDLL(so_path)
    if not hasattr(lib, "axon_start_nrt_profile"):
        return
    lib.axon_start_nrt_profile.argtypes = [
        ctypes.POINTER(ctypes.c_int64),
        ctypes.c_size_t,
    ]
    lib.axon_start_nrt_profile.restype = ctypes.c_int64
    lib.axon_stop_nrt_profile.argtypes = [ctypes.c_char_p]
    lib.axon_stop_nrt_profile.restype = ctypes.c_int64

    @contextlib.contextmanager
    def _hook(output_dir, device_ids):
        import jax

        jax.devices()
        if device_ids:
            ids = (ctypes.c_int64 * len(device_ids))(*device_ids)
            rc = lib.axon_start_nrt_profile(ids, len(device_ids))
        else:
            rc = lib.axon_start_nrt_profile(None, 0)
        if rc != 0:
            raise RuntimeError(f"axon_start_nrt_profile rc={rc}")
        try:
            yield
        finally:
            n = lib.axon_stop_nrt_profile(str(output_dir).encode())
            print(f"ntff profile: {n} file(s) -> {output_dir}", file=sys.stderr)

    mod.set_axon_ntff_profile_hook(_hook)

    # No artifact bucket in this container; keep profiles local.
    import concourse.bass_utils as bu

    bu.upload_artifacts = lambda tmpdir: tmpdir


def kernel(x, attention, row_weights, col_weights, _trace=False):
    from concourse.bass_utils import run_bass_kernel_spmd

    x = np.ascontiguousarray(x, dtype=np.float32)
    m_row, m_col = _weight_matrices(attention, row_weights, col_weights)

    if _trace:
        _install_axon_ntff_shim()
    nc = _get_nc()
    in_maps = [
        {
            "x": x[i * B_PER_CORE : (i + 1) * B_PER_CORE],
            "mrow": m_row,
            "mcol": m_col,
        }
        for i in range(N_CORES)
    ]
    res = run_bass_kernel_spmd(
        nc, in_maps, core_ids=list(range(N_CORES)), trace=_trace
    )
    out = np.concatenate([res.results[i]["out"] for i in range(N_CORES)], axis=0)
    if _trace:
        kernel.last_exec_time_ns = res.exec_time_ns
        kernel.last_results = res
    return out


# revision 12
# speedup vs baseline: 1.0176x; 1.0176x over previous
"""Trainium2 Bass kernel for nn_FMCTracker.

Reference computation:
    xd   = dct2(x)                 # 2D DCT-II (ortho) over (H, W)
    gray = xd.mean(axis=1)         # channel mean
    w    = gray * attention
    row_out[b] = mean_h(rw[h] * mean_w(w[b,h,w]))
    col_out[b] = mean_w(cw[w] * mean_h(w[b,h,w]))
    out = sigmoid(stack([col_out, row_out], axis=1))   # [B, 2]

Everything between x and the sigmoid is linear in x, so the whole chain
folds into two fixed 512x512 weighting matrices (computed on the host from
the tiny attention/row/col weights):

    M_row = D^T @ (diag(rw) @ att) @ D / (C*H*W)
    M_col = D^T @ (att @ diag(cw)) @ D / (C*H*W)
    row_out[b] = sum_{c,i,j} x[b,c,i,j] * M_row[i,j]

The device kernel is then a pure memory-bound weighted reduction over x
(201 MB), data-parallel over batch across 8 NeuronCores (8 images each).

Per core, per image b:
    DMA   x[b] (3 MB fp32) -> SBUF as [128p, c, k, 512] (i = k*128 + p)
    DVE   S = c0 + c1 + c2              (two fp32 tensor_adds, bf16 out)
    DVE   tensor_tensor_reduce: (S * M_col) -> accum [128,1] fp32
    DVE   tensor_tensor_reduce: (S * M_row) -> accum [128,1] fp32
    (accum scalars collect into acc[128, 16], one column per (b, output))
Tail: one 128x16 matmul vs ones reduces partitions -> [16,1] PSUM,
ScalarE sigmoid, DMA 64 B out.
"""

import os
from contextlib import ExitStack

import numpy as np
import ml_dtypes

B, C, H, W = 64, 3, 512, 512
N_CORES = 8
B_PER_CORE = B // N_CORES  # 8


def _dct_matrix(n: int) -> np.ndarray:
    """DCT-II orthonormal matrix D, so that dct(v, norm='ortho') = D @ v."""
    i = np.arange(n)
    k = np.arange(n)[:, None]
    d = np.cos(np.pi * (2 * i[None, :] + 1) * k / (2 * n))
    coef = np.full((n, 1), np.sqrt(2.0 / n))
    coef[0] = np.sqrt(1.0 / n)
    return d * coef


def _weight_matrices(attention, row_weights, col_weights):
    """Fold DCT + all downstream linear reductions into two [H, W] matrices."""
    d = _dct_matrix(H)
    att = attention.astype(np.float64)
    rw = row_weights.astype(np.float64)
    cw = col_weights.astype(np.float64)
    scale = 1.0 / (C * H * W)
    m_row = d.T @ (rw[:, None] * att) @ d * scale
    m_col = d.T @ (att * cw[None, :]) @ d * scale
    return (
        m_row.astype(ml_dtypes.bfloat16),
        m_col.astype(ml_dtypes.bfloat16),
    )


_NC_CACHE = {}


def _build_bass():
    import concourse.bacc as bacc
    import concourse.tile as tile
    from concourse import mybir

    nc = bacc.Bacc(
        "TRN2", target_bir_lowering=False, debug=False, num_devices=N_CORES
    )
    x_in = nc.declare_dram_parameter(
        "x", [B_PER_CORE, C, H, W], mybir.dt.float32, isOutput=False
    )
    mrow_in = nc.declare_dram_parameter(
        "mrow", [H, W], mybir.dt.bfloat16, isOutput=False
    )
    mcol_in = nc.declare_dram_parameter(
        "mcol", [H, W], mybir.dt.bfloat16, isOutput=False
    )
    out_ext = nc.declare_dram_parameter(
        "out", [B_PER_CORE, 2], mybir.dt.float32, isOutput=True
    )

    P = 128
    K = H // P  # 4 chunks of rows per plane

    with tile.TileContext(nc) as tc:
        with ExitStack() as ctx:
            singles = ctx.enter_context(tc.tile_pool(name="singles", bufs=1))
            xpool = ctx.enter_context(tc.tile_pool(name="xpool", bufs=3))
            spool = ctx.enter_context(tc.tile_pool(name="spool", bufs=3))
            psum = ctx.enter_context(tc.tile_pool(name="psum", bufs=1, space="PSUM"))

            # Alternate the two HWDGE rings (SP + ACT) so consecutive x-plane
            # DMAs overlap their fixed completion latencies.
            dma_engines = [nc.sync, nc.scalar]
            n_dma = 0

            def xdma(out, in_):
                nonlocal n_dma
                dma_engines[n_dma % 2].dma_start(out=out, in_=in_)
                n_dma += 1

            ones_sb = singles.tile([P, 1], mybir.dt.float32)
            nc.vector.memset(ones_sb, 1.0)
            acc = singles.tile([P, 2 * B_PER_CORE], mybir.dt.float32)
            mrow_sb = singles.tile([P, K, W], mybir.dt.bfloat16)
            mcol_sb = singles.tile([P, K, W], mybir.dt.bfloat16)

            for b in range(B_PER_CORE):
                # per-plane tiles so the first add only waits on c0+c1
                xc = [
                    xpool.tile(
                        [P, K, W], mybir.dt.float32, name=f"xc{c}", tag=f"xc{c}"
                    )
                    for c in range(C)
                ]
                for c in range(C):
                    xdma(xc[c], x_in[b, c].rearrange("(k p) j -> p k j", p=P))
                if b == 0:
                    # defer the small weight loads behind the first image's
                    # planes to shorten the pipeline lead-in
                    nc.sync.dma_start(
                        out=mcol_sb,
                        in_=mcol_in.rearrange("(k p) j -> p k j", p=P),
                    )
                    nc.scalar.dma_start(
                        out=mrow_sb,
                        in_=mrow_in.rearrange("(k p) j -> p k j", p=P),
                    )
                t01 = spool.tile([P, K, W], mybir.dt.float32, tag="t01")
                nc.vector.tensor_add(t01, xc[0], xc[1])
                s_bf = spool.tile([P, K, W], mybir.dt.bfloat16, tag="s_bf")
                nc.vector.tensor_add(s_bf, t01, xc[2])
                for e, m_sb in ((0, mcol_sb), (1, mrow_sb)):
                    prod = spool.tile(
                        [P, K, W], mybir.dt.bfloat16, name="prod", tag=f"prod{e}"
                    )
                    nc.vector.tensor_mul(prod, s_bf, m_sb)
                    scr = spool.tile(
                        [P, K, W], mybir.dt.bfloat16, name="scr", tag=f"scr{e}"
                    )
                    nc.scalar.activation(
                        out=scr,
                        in_=prod,
                        func=mybir.ActivationFunctionType.Copy,
                        accum_out=acc[:, 2 * b + e : 2 * b + e + 1],
                    )

            ps = psum.tile([2 * B_PER_CORE, 1], mybir.dt.float32)
            nc.tensor.matmul(
                out=ps, lhsT=acc, rhs=ones_sb, start=True, stop=True
            )
            out_sb = singles.tile([2 * B_PER_CORE, 1], mybir.dt.float32)
            nc.scalar.activation(
                out=out_sb, in_=ps, func=mybir.ActivationFunctionType.Sigmoid
            )
            nc.sync.dma_start(
                out=out_ext.rearrange("b e -> (b e) ()"), in_=out_sb
            )
    nc.compile()
    return nc


def _get_nc():
    if "nc" not in _NC_CACHE:
        _NC_CACHE["nc"] = _build_bass()
    return _NC_CACHE["nc"]


def _install_axon_ntff_shim():
    """Provide antenv.axon_hooks (NTFF profiling hook) when the image lacks it.

    Mirrors trn_agent_boot's ctypes hook against libaxon_pjrt.so so that
    run_bass_kernel_spmd(trace=True) can capture NTFF profiles under axon.
    """
    import sys
    import types
    import ctypes
    import contextlib

    try:
        from antenv.axon_hooks import get_axon_ntff_profile_hook  # noqa: F401

        return
    except ImportError:
        pass

    import antenv

    mod = types.ModuleType("antenv.axon_hooks")
    _state = {"hook": None}
    mod.set_axon_ntff_profile_hook = lambda h: _state.__setitem__("hook", h)
    mod.get_axon_ntff_profile_hook = lambda: _state["hook"]
    antenv.axon_hooks = mod
    sys.modules["antenv.axon_hooks"] = mod

    so_path = "/opt/axon/libaxon_pjrt.so"
    if not os.path.exists(so_path):
        return
    lib = ctypes.CDLL(so_path)
    if not hasattr(lib, "axon_start_nrt_profile"):
        return
    lib.axon_start_nrt_profile.argtypes = [
        ctypes.POINTER(ctypes.c_int64),
        ctypes.c_size_t,
    ]
    lib.axon_start_nrt_profile.restype = ctypes.c_int64
    lib.axon_stop_nrt_profile.argtypes = [ctypes.c_char_p]
    lib.axon_stop_nrt_profile.restype = ctypes.c_int64

    @contextlib.contextmanager
    def _hook(output_dir, device_ids):
        import jax

        jax.devices()
        if device_ids:
            ids = (ctypes.c_int64 * len(device_ids))(*device_ids)
            rc = lib.axon_start_nrt_profile(ids, len(device_ids))
        else:
            rc = lib.axon_start_nrt_profile(None, 0)
        if rc != 0:
            raise RuntimeError(f"axon_start_nrt_profile rc={rc}")
        try:
            yield
        finally:
            n = lib.axon_stop_nrt_profile(str(output_dir).encode())
            print(f"ntff profile: {n} file(s) -> {output_dir}", file=sys.stderr)

    mod.set_axon_ntff_profile_hook(_hook)

    # No artifact bucket in this container; keep profiles local.
    import concourse.bass_utils as bu

    bu.upload_artifacts = lambda tmpdir: tmpdir


def kernel(x, attention, row_weights, col_weights, _trace=False):
    from concourse.bass_utils import run_bass_kernel_spmd

    x = np.ascontiguousarray(x, dtype=np.float32)
    m_row, m_col = _weight_matrices(attention, row_weights, col_weights)

    if _trace:
        _install_axon_ntff_shim()
    nc = _get_nc()
    in_maps = [
        {
            "x": x[i * B_PER_CORE : (i + 1) * B_PER_CORE],
            "mrow": m_row,
            "mcol": m_col,
        }
        for i in range(N_CORES)
    ]
    res = run_bass_kernel_spmd(
        nc, in_maps, core_ids=list(range(N_CORES)), trace=_trace
    )
    out = np.concatenate([res.results[i]["out"] for i in range(N_CORES)], axis=0)
    if _trace:
        kernel.last_exec_time_ns = res.exec_time_ns
        kernel.last_results = res
    return out
